# revision 7
# baseline (speedup 1.0000x reference)
"""DeepseekV2 MLA attention prefill kernel for 8 Trainium2 NeuronCores.

Sharding: 2-way data-parallel over batch x 4-way tensor-parallel over heads
(4 heads per core).  Both the q down-projection and the compressed-KV
projection are computed on the core's S/4 local slice and exchanged with
two in-group AllGathers (ckv+kpe 590KB first, q_norm 1.5MB second); KV
decompression consumes the gathered ckv while the q gather is in flight.
Per-head up-projections, attention and the output projection are computed
locally; o_proj partial sums are written bf16 and reduced on the host
during unsharding.

Layouts: activations are feature-major ([D, S]) throughout; attention
scores are computed transposed ([s_k, s_q]) so the PV matmul needs no
transposes.  RoPE is applied via host-side permuted/sign-folded weight
columns.  Matmuls run in bf16 with fp32 PSUM accumulation.  Per-column
scale vectors (RMSNorm rstd, softmax 1/sum) are broadcast across
partitions with a K=1 ones-matmul on TensorE (GpSimd runs only the two
collectives, so nothing queues behind them).  The softmax reciprocal is
computed on VectorE (reciprocal_approx_fast), keeping ScalarE inside the
Exp activation table for the whole attention phase.  Weights are loaded
with few large DMAs; o_proj is interleaved per seq chunk with staged
writeback.
"""
import sys
sys.path.insert(0, "/opt/trn_rl_repo")

import math
import numpy as np
import ml_dtypes

import concourse.bass as bass
import concourse.tile as tile
from concourse import bacc, mybir
from concourse.bass_utils import run_bass_kernel_spmd

# ---- problem constants (hardcoded; kernel.py must be self-contained) ----
B, S, HID, H = 2, 2048, 2048, 16
Q_LORA, KV_LORA = 1536, 512
D_NOPE, D_ROPE, D_V = 128, 64, 128
D_Q = D_NOPE + D_ROPE
EPS = 1e-6
ROPE_THETA = 10000.0
N_CORES = 8
HPC = 4                      # heads per core
GROUPS = [[0, 1, 2, 3], [4, 5, 6, 7]]

F32 = mybir.dt.float32
BF16 = mybir.dt.bfloat16
MM_DT = BF16

SCALE = 1.0 / math.sqrt(D_Q)

_CACHE = {}

KC = HID // 128              # 16 contraction tiles for HID
QKC = Q_LORA // 128          # 12 contraction tiles for Q_LORA
CKC = KV_LORA // 128         # 4 contraction tiles for KV_LORA
CKV_G = KV_LORA + D_ROPE     # 576 gathered rows (ckv | roped kpe)


# ---------------------------------------------------------------- builder --
def build_kernel(mm_dt=MM_DT):
    nc = bacc.Bacc("TRN2", target_bir_lowering=False, debug=False,
                   num_devices=N_CORES)

    xt_loc = nc.dram_tensor("xt_loc", [HID, 512], mm_dt, kind="ExternalInput")
    wdq = nc.dram_tensor("wdq", [HID, Q_LORA], mm_dt, kind="ExternalInput")
    wuq = nc.dram_tensor("wuq", [Q_LORA, HPC * 256], mm_dt, kind="ExternalInput")
    wkva = nc.dram_tensor("wkva", [HID, KV_LORA + 2 * D_ROPE], mm_dt, kind="ExternalInput")
    wkvb = nc.dram_tensor("wkvb", [KV_LORA, HPC, 256], mm_dt, kind="ExternalInput")
    ow = nc.dram_tensor("ow", [HPC, D_V, HID], mm_dt, kind="ExternalInput")
    cos_f = nc.dram_tensor("cos_f", [D_ROPE, S], mm_dt, kind="ExternalInput")
    sin_f = nc.dram_tensor("sin_f", [D_ROPE, S], mm_dt, kind="ExternalInput")
    cos_l = nc.dram_tensor("cos_l", [D_ROPE, 512], mm_dt, kind="ExternalInput")
    sin_l = nc.dram_tensor("sin_l", [D_ROPE, 512], mm_dt, kind="ExternalInput")
    tri = nc.dram_tensor("tri", [128, 128], mm_dt, kind="ExternalInput")
    out_t = nc.dram_tensor("out_t", [HID, S], mm_dt, kind="ExternalOutput")

    with tile.TileContext(nc) as tc:
        import contextlib
        ctx = contextlib.ExitStack()
        with ctx:
            persist = ctx.enter_context(tc.tile_pool(name="persist", bufs=1))
            wpool = ctx.enter_context(tc.tile_pool(name="wpool", bufs=4))
            spool = ctx.enter_context(tc.tile_pool(name="spool", bufs=2))
            xpool = ctx.enter_context(tc.tile_pool(name="xpool", bufs=4))
            ppool = ctx.enter_context(tc.tile_pool(name="ppool", bufs=2, space="PSUM"))
            pscore = ctx.enter_context(tc.tile_pool(name="pscore", bufs=3, space="PSUM"))
            pctx = ctx.enter_context(tc.tile_pool(name="pctx", bufs=2, space="PSUM"))
            psums = ctx.enter_context(tc.tile_pool(name="psums", bufs=1, space="PSUM"))
            dram = ctx.enter_context(tc.tile_pool(name="dram", bufs=1, space="DRAM"))

            # ---- first-need-order DMA loads -------------------------------
            # stage A gate: x local slice + kv_a weights + local rope tables
            xl_sb = []
            for j in range(4):
                t = xpool.tile([128, 4, 512], mm_dt, tag="xl")
                nc.sync.dma_start(
                    out=t,
                    in_=xt_loc.ap()[j * 512:(j + 1) * 512, :]
                    .rearrange("(kc p) n -> p kc n", p=128))
                xl_sb.append(t)
            wkva_sb = persist.tile([128, KC, 640], mm_dt, tag="wkva")
            nc.sync.dma_start(out=wkva_sb,
                              in_=wkva.ap().rearrange("(kc p) c -> p kc c", p=128))
            cosl_sb = persist.tile([D_ROPE, 512], mm_dt, tag="cosl")
            sinl_sb = persist.tile([D_ROPE, 512], mm_dt, tag="sinl")
            nc.sync.dma_start(out=cosl_sb, in_=cos_l.ap())
            nc.sync.dma_start(out=sinl_sb, in_=sin_l.ap())

            # stage B gate: q down-proj weights (first group)
            def load_wdq(mg):
                out = []
                for j in range(4):
                    t = wpool.tile([128, 4, 512], mm_dt, tag="wdq")
                    nc.sync.dma_start(
                        out=t,
                        in_=wdq.ap()[j * 512:(j + 1) * 512,
                                     mg * 512:(mg + 1) * 512]
                        .rearrange("(kc p) n -> p kc n", p=128))
                    out.append(t)
                return out

            wdq_sb = {0: load_wdq(0)}

            # constants
            ones_sb = persist.tile([128, 1], mm_dt, tag="ones")
            nc.vector.memset(ones_sb, 1.0)
            ones_row = persist.tile([1, 128], mm_dt, tag="ones_row")
            nc.vector.memset(ones_row, 1.0)
            eps_sb = persist.tile([1, 1], F32, tag="eps")
            nc.vector.memset(eps_sb, EPS)

            # later-stage persistent loads (queue drains in background)
            wkvb_sb = persist.tile([128, CKC, HPC, 256], mm_dt, tag="wkvb")
            nc.sync.dma_start(out=wkvb_sb,
                              in_=wkvb.ap().rearrange("(kc p) h c -> p kc h c", p=128))
            cosf_sb = persist.tile([D_ROPE, 4, 512], mm_dt, tag="cosf")
            sinf_sb = persist.tile([D_ROPE, 4, 512], mm_dt, tag="sinf")
            nc.sync.dma_start(out=cosf_sb, in_=cos_f.ap().rearrange("d (c n) -> d c n", c=4))
            nc.sync.dma_start(out=sinf_sb, in_=sin_f.ap().rearrange("d (c n) -> d c n", c=4))
            tri_sb = persist.tile([128, 128], mm_dt, tag="tri")
            nc.sync.dma_start(out=tri_sb, in_=tri.ap())
            ow_sb = persist.tile([D_V, HPC, HID], mm_dt, tag="ow")
            nc.sync.dma_start(out=ow_sb, in_=ow.ap().rearrange("h p c -> p h c"))

            # gather buffers (DRAM)
            g_in_ckv = dram.tile([CKV_G, 512], mm_dt, name="g_in_ckv")
            g_out_ckv = dram.tile([4 * CKV_G, 512], mm_dt, name="g_out_ckv")
            g_in_q = dram.tile([Q_LORA, 512], mm_dt, name="g_in_q")
            g_out_q = dram.tile([4 * Q_LORA, 512], mm_dt, name="g_out_q")

            # column-broadcast helper: [1,512] f32 -> [128,512] f32 (PSUM)
            # via a K=1 ones-matmul on TensorE (keeps GpSimd free).
            def col_broadcast(vec_f32, pool, tag):
                vb = spool.tile([1, 512], mm_dt, tag="vecbf", bufs=2)
                nc.scalar.copy(vb, vec_f32)
                bc = pool.tile([128, 512], F32, tag=tag, name="bc")
                nc.tensor.matmul(bc, ones_row, vb, start=True, stop=True,
                                 skip_group_check=True)
                return bc

            # ---- stage A: local-slice compressed KV + rope + RMSNorm ------
            ckv_loc = spool.tile([128, CKC, 512], mm_dt, tag="ckv", bufs=1)
            kpe_loc = spool.tile([D_ROPE, 512], mm_dt, tag="kpe_loc", bufs=1)
            ssq_kv = psums.tile([1, 512], F32, tag="p_sum", name="ssq_kv")
            accs = [ppool.tile([128, 512], F32, tag="p_a", name="acc_kv")
                    if j < 2 else
                    pscore.tile([128, 512], F32, tag="p_sc", name="acc_kv2")
                    for j in range(5)]
            for k in range(KC):
                for j in range(5):
                    nc.tensor.matmul(
                        accs[j], wkva_sb[:, k, j * 128:(j + 1) * 128],
                        xl_sb[k // 4][:, k % 4, :],
                        start=(k == 0), stop=(k == KC - 1))
            for j in range(CKC):
                nc.vector.tensor_copy(ckv_loc[:, j, :], accs[j])
                sq = spool.tile([128, 512], mm_dt, tag="sq", bufs=1)
                nc.vector.tensor_tensor(sq, ckv_loc[:, j, :], ckv_loc[:, j, :],
                                        mybir.AluOpType.mult)
                nc.tensor.matmul(ssq_kv, ones_sb, sq,
                                 start=(j == 0), stop=(j == CKC - 1),
                                 skip_group_check=True)
            # rope chunk [E(64) | R(64)] -> kpe_loc
            t0 = spool.tile([D_ROPE, 512], F32, tag="ropet0", bufs=1)
            t1 = spool.tile([D_ROPE, 512], F32, tag="ropet1", bufs=1)
            nc.vector.tensor_tensor(t0, accs[4][0:D_ROPE, :], cosl_sb,
                                    mybir.AluOpType.mult)
            nc.vector.tensor_tensor(t1, accs[4][D_ROPE:2 * D_ROPE, :], sinl_sb,
                                    mybir.AluOpType.mult)
            nc.vector.tensor_tensor(kpe_loc, t0, t1, mybir.AluOpType.add)
            # rstd = exp(-0.5 ln(ms+eps)); broadcast on TensorE
            ls = spool.tile([1, 512], F32, tag="lsum", bufs=1)
            nc.scalar.activation(out=ls, in_=ssq_kv,
                                 func=mybir.ActivationFunctionType.Ln,
                                 bias=eps_sb, scale=1.0 / KV_LORA)
            rstd = spool.tile([1, 512], F32, tag="rstd", bufs=1)
            nc.scalar.activation(out=rstd, in_=ls, scale=-0.5,
                                 func=mybir.ActivationFunctionType.Exp)
            rstd_bc = col_broadcast(rstd, ppool, "p_a")
            for j in range(CKC):
                nc.vector.tensor_tensor(ckv_loc[:, j, :], ckv_loc[:, j, :],
                                        rstd_bc, mybir.AluOpType.mult)
            nc.sync.dma_start(
                out=g_in_ckv[0:KV_LORA, :].rearrange("(m p) n -> p m n", p=128),
                in_=ckv_loc)
            nc.sync.dma_start(out=g_in_ckv[KV_LORA:CKV_G, :], in_=kpe_loc)
            nc.gpsimd.collective_compute(
                "AllGather", mybir.AluOpType.bypass,
                replica_groups=GROUPS,
                ins=[g_in_ckv.opt()], outs=[g_out_ckv.opt()])

            # ---- stage B: q down-proj + RMSNorm on the local S chunk ------
            qnorm_own = spool.tile([128, QKC, 512], mm_dt, tag="qnorm_own", bufs=1)
            ssq_q = psums.tile([1, 512], F32, tag="p_sum", name="ssq_q")
            for mg in range(3):
                if mg not in wdq_sb:
                    wdq_sb[mg] = load_wdq(mg)
                wts = wdq_sb[mg]
                accs = [ppool.tile([128, 512], F32, tag="p_a", name="acc_q")
                        if j < 2 else
                        pscore.tile([128, 512], F32, tag="p_sc", name="acc_q2")
                        for j in range(4)]
                for k in range(KC):
                    for j in range(4):
                        nc.tensor.matmul(
                            accs[j], wts[k // 4][:, k % 4, j * 128:(j + 1) * 128],
                            xl_sb[k // 4][:, k % 4, :],
                            start=(k == 0), stop=(k == KC - 1))
                for j in range(4):
                    m = mg * 4 + j
                    nc.vector.tensor_copy(qnorm_own[:, m, :], accs[j])
                    sq = spool.tile([128, 512], mm_dt, tag="sq", bufs=1)
                    nc.vector.tensor_tensor(sq, qnorm_own[:, m, :], qnorm_own[:, m, :],
                                            mybir.AluOpType.mult)
                    nc.tensor.matmul(ssq_q, ones_sb, sq,
                                     start=(m == 0), stop=(m == QKC - 1),
                                     skip_group_check=True)
            ls2 = spool.tile([1, 512], F32, tag="lsum", bufs=1)
            nc.scalar.activation(out=ls2, in_=ssq_q,
                                 func=mybir.ActivationFunctionType.Ln,
                                 bias=eps_sb, scale=1.0 / Q_LORA)
            rstd2 = spool.tile([1, 512], F32, tag="rstd", bufs=1)
            nc.scalar.activation(out=rstd2, in_=ls2, scale=-0.5,
                                 func=mybir.ActivationFunctionType.Exp)
            rstd2_bc = col_broadcast(rstd2, ppool, "p_a")
            for m in range(QKC):
                nc.vector.tensor_tensor(qnorm_own[:, m, :], qnorm_own[:, m, :],
                                        rstd2_bc, mybir.AluOpType.mult)
            nc.sync.dma_start(
                out=g_in_q.rearrange("(m p) n -> p m n", p=128),
                in_=qnorm_own)
            nc.gpsimd.collective_compute(
                "AllGather", mybir.AluOpType.bypass,
                replica_groups=GROUPS,
                ins=[g_in_q.opt()], outs=[g_out_q.opt()])

            # ---- stage C: KV decompression from gathered ckv --------------
            kpe_sb = persist.tile([D_ROPE, 4, 512], mm_dt, tag="kpe")
            kn_sb = persist.tile([D_NOPE, HPC, 4, 512], mm_dt, tag="kn")
            v_sb = persist.tile([128, S // 128, HPC * D_V], mm_dt, tag="v")

            for nch in range(4):
                ckv_g = xpool.tile([128, CKC, 512], mm_dt, tag="xl")
                nc.sync.dma_start(
                    out=ckv_g,
                    in_=g_out_ckv[CKV_G * nch:CKV_G * nch + KV_LORA, :]
                    .rearrange("(m p) n -> p m n", p=128))
                nc.sync.dma_start(
                    out=kpe_sb[:, nch, :],
                    in_=g_out_ckv[CKV_G * nch + KV_LORA:CKV_G * (nch + 1), :])
                for h in range(HPC):
                    acc = ppool.tile([128, 512], F32, tag="p_a", name="acc_kn")
                    for k in range(CKC):
                        nc.tensor.matmul(acc, wkvb_sb[:, k, h, 0:128],
                                         ckv_g[:, k, :],
                                         start=(k == 0), stop=(k == CKC - 1))
                    nc.scalar.copy(kn_sb[:, h, nch, :], acc)
                for st in range(4):
                    skt = nch * 4 + st
                    acc = ppool.tile([128, 512], F32, tag="p_a", name="acc_v")
                    for k in range(CKC):
                        nc.tensor.matmul(
                            acc, ckv_g[:, k, st * 128:(st + 1) * 128],
                            wkvb_sb[:, k, :, 128:256],
                            start=(k == 0), stop=(k == CKC - 1))
                    nc.scalar.copy(v_sb[:, skt, :], acc)

            # ---- stage D: per-seq-chunk q up-proj + attn + o_proj ---------
            for sqc in range(4):
                # stream this chunk's q_norm (post-gather) in 3 thirds
                qn_src = []
                for t in range(3):
                    qf = wpool.tile([128, 4, 512], mm_dt, tag="wdq")
                    nc.sync.dma_start(
                        out=qf,
                        in_=g_out_q[Q_LORA * sqc + 512 * t:
                                         Q_LORA * sqc + 512 * (t + 1), :]
                        .rearrange("(m p) n -> p m n", p=128))
                    qn_src.append(qf)

                qn_t = {}
                qpe_t = {}
                for g2 in range(HPC):   # one head (nope + rope chunk) per pass
                    wuq_s = spool.tile([128, QKC, 256], mm_dt, tag="wuq_s", bufs=2)
                    nc.sync.dma_start(
                        out=wuq_s,
                        in_=wuq.ap()[:, g2 * 256:(g2 + 1) * 256]
                        .rearrange("(kc p) c -> p kc c", p=128))
                    accs = [ppool.tile([128, 512], F32, tag="p_a", name="acc_qup")
                            for _ in range(2)]
                    for k in range(QKC):
                        for j in range(2):
                            nc.tensor.matmul(
                                accs[j],
                                wuq_s[:, k, j * 128:(j + 1) * 128],
                                qn_src[k // 4][:, k % 4, :],
                                start=(k == 0), stop=(k == QKC - 1))
                    h = g2
                    qt = spool.tile([D_NOPE, 512], mm_dt, tag="qn_h%d" % h, bufs=1)
                    nc.scalar.copy(qt, accs[0])
                    qn_t[h] = qt
                    t0 = spool.tile([D_ROPE, 512], F32, tag="ropet0", bufs=1)
                    t1 = spool.tile([D_ROPE, 512], F32, tag="ropet1", bufs=1)
                    nc.vector.tensor_tensor(t0, accs[1][0:D_ROPE, :],
                                            cosf_sb[:, sqc, :], mybir.AluOpType.mult)
                    nc.vector.tensor_tensor(t1, accs[1][D_ROPE:2 * D_ROPE, :],
                                            sinf_sb[:, sqc, :], mybir.AluOpType.mult)
                    qpt = spool.tile([D_ROPE, 512], mm_dt, tag="qpe_h%d" % h, bufs=1)
                    nc.vector.tensor_tensor(qpt, t0, t1, mybir.AluOpType.add)
                    qpe_t[h] = qpt

                n_skt = 4 * (sqc + 1)
                ctx_sb = spool.tile([D_V, HPC, 512], mm_dt, tag="ctx", bufs=1)
                fin_pend = None   # (h, sum_acc, ctx_acc): finalize 1 head behind

                def finalize(fh, fsum, fctx):
                    # 1/sum on VectorE; broadcast on TensorE; scale on VectorE
                    sf = spool.tile([1, 512], F32, tag="sumf", bufs=2)
                    nc.scalar.copy(sf, fsum)
                    rc = spool.tile([1, 512], F32, tag="recip", bufs=2)
                    nc.vector.reciprocal_approx_fast(out=rc, in_=sf)
                    rc_bc = col_broadcast(rc, ppool, "p_a")
                    # DVE may read only one PSUM operand: stage bcast in SBUF
                    rc_sb = spool.tile([128, 512], mm_dt, tag="rc_sb", bufs=2)
                    nc.scalar.copy(rc_sb, rc_bc)
                    nc.vector.tensor_tensor(ctx_sb[:, fh, :], fctx, rc_sb,
                                            mybir.AluOpType.mult)

                for h in range(HPC):
                    sum_acc = psums.tile([1, 512], F32, tag="p_sum", name="sum_acc")
                    ctx_acc = pctx.tile([D_V, 512], F32, tag="p_ctx")
                    pending = []   # pipeline: exp tiles awaiting sums/PV
                    for skt in range(n_skt):
                        # diagonal chunk dd: columns < 128*dd are fully masked
                        # — compute only the causal column slice
                        dd = skt - 4 * sqc
                        c0 = 128 * dd if dd > 0 else 0
                        sc = pscore.tile([128, 512], F32, tag="p_sc", name="sc")
                        nc.tensor.matmul(
                            sc[:, c0:],
                            kn_sb[:, h, skt // 4, (skt % 4) * 128:(skt % 4) * 128 + 128],
                            qn_t[h][:, c0:], start=True, stop=False,
                            skip_group_check=True)
                        nc.tensor.matmul(
                            sc[:, c0:],
                            kpe_sb[:, skt // 4, (skt % 4) * 128:(skt % 4) * 128 + 128],
                            qpe_t[h][:, c0:], start=False, stop=True,
                            skip_group_check=True)
                        ex = spool.tile([128, 512], mm_dt, tag="exp%d" % (skt % 4), bufs=1)
                        nc.scalar.activation(out=ex[:, c0:], in_=sc[:, c0:],
                                             func=mybir.ActivationFunctionType.Exp,
                                             scale=SCALE)
                        if dd >= 0:
                            # only the first 128 columns of the slice touch the
                            # causal boundary — mask just that triangle
                            nc.vector.tensor_tensor(ex[:, c0:c0 + 128],
                                                    ex[:, c0:c0 + 128],
                                                    tri_sb, mybir.AluOpType.mult)
                        pending.append((ex, skt, c0))
                        if len(pending) > 3:
                            pex, pskt, pc0 = pending.pop(0)
                            nc.tensor.matmul(sum_acc[:, pc0:], ones_sb, pex[:, pc0:],
                                             start=(pskt == 0), stop=False,
                                             skip_group_check=True)
                            nc.tensor.matmul(ctx_acc[:, pc0:],
                                             v_sb[:, pskt, h * D_V:(h + 1) * D_V],
                                             pex[:, pc0:], start=(pskt == 0), stop=False,
                                             skip_group_check=True)
                        if skt == 1 and fin_pend is not None:
                            finalize(*fin_pend)
                            fin_pend = None
                    while pending:
                        pex, pskt, pc0 = pending.pop(0)
                        last = not pending
                        nc.tensor.matmul(sum_acc[:, pc0:], ones_sb, pex[:, pc0:],
                                         start=(pskt == 0), stop=last,
                                         skip_group_check=True)
                        nc.tensor.matmul(ctx_acc[:, pc0:],
                                         v_sb[:, pskt, h * D_V:(h + 1) * D_V],
                                         pex[:, pc0:], start=(pskt == 0), stop=last,
                                         skip_group_check=True)
                    fin_pend = (h, sum_acc, ctx_acc)
                finalize(*fin_pend)
                fin_pend = None

                # o_proj for this seq chunk (partial sums over local heads)
                for og in range(4):
                    ostage = spool.tile([128, 4, 512], mm_dt, tag="ostage", bufs=1)
                    for hc in range(4):
                        hidc = og * 4 + hc
                        acc = ppool.tile([128, 512], F32, tag="p_a", name="acc_o")
                        for h in range(HPC):
                            nc.tensor.matmul(acc, ow_sb[:, h, hidc * 128:(hidc + 1) * 128],
                                             ctx_sb[:, h, :],
                                             start=(h == 0), stop=(h == HPC - 1))
                        nc.scalar.copy(ostage[:, hc, :], acc)
                    nc.sync.dma_start(
                        out=out_t.ap()[og * 512:(og + 1) * 512,
                                       sqc * 512:(sqc + 1) * 512]
                        .rearrange("(hc p) n -> p hc n", p=128),
                        in_=ostage)

    nc.compile()
    return nc


# ------------------------------------------------------------- host side --
def _rope_tables():
    inv_freq = 1.0 / (ROPE_THETA ** (np.arange(0, D_ROPE, 2, dtype=np.float64) / D_ROPE))
    t = np.arange(S, dtype=np.float64)
    freqs = np.outer(t, inv_freq)                    # [S, 32]
    emb = np.concatenate([freqs, freqs], axis=-1)    # [S, 64]
    return (np.cos(emb).astype(np.float32).T.copy(),
            np.sin(emb).astype(np.float32).T.copy())  # [64, S]


_E_PERM = np.concatenate([np.arange(0, D_ROPE, 2), np.arange(1, D_ROPE, 2)])


def _rope_expand(Wpe):
    """[n, 64] rope weight cols -> [n, 128]: [even/odd-reordered | rot-half signed]."""
    Y = Wpe[:, _E_PERM]
    R = np.concatenate([-Y[:, D_ROPE // 2:], Y[:, :D_ROPE // 2]], axis=1)
    return np.concatenate([Y, R], axis=1)


def _prep_inputs(hidden_states, w_dq, q_a_ln_w, w_uq, kv_a_w, kv_a_ln_w, kv_b_w, o_w):
    bf = ml_dtypes.bfloat16
    cosT, sinT = _rope_tables()

    wuq_eff = (np.asarray(q_a_ln_w)[:, None] * np.asarray(w_uq)).reshape(Q_LORA, H, D_Q)
    head_blocks = []
    for h in range(H):
        head_blocks.append(np.concatenate(
            [wuq_eff[:, h, :D_NOPE], _rope_expand(wuq_eff[:, h, D_NOPE:])], axis=1))
    wuq_x = np.stack(head_blocks, axis=1)            # [1536, 16, 256]

    kv_a = np.asarray(kv_a_w)
    wkva_x = np.concatenate([kv_a[:, :KV_LORA], _rope_expand(kv_a[:, KV_LORA:])],
                            axis=1).astype(bf)       # [2048, 640]
    wkvb_eff = (np.asarray(kv_a_ln_w)[:, None] * np.asarray(kv_b_w)).reshape(KV_LORA, H, 256)
    ow_r = np.asarray(o_w).reshape(H, D_V, HID)

    tri = (np.arange(128)[None, :] >= np.arange(128)[:, None]).astype(bf)

    wdq_b = np.asarray(w_dq).astype(bf)
    hs = np.asarray(hidden_states)

    in_maps = []
    for c in range(N_CORES):
        b, hg = c // 4, c % 4
        s0 = 512 * hg
        xt_loc = np.ascontiguousarray(hs[b].T[:, s0:s0 + 512]).astype(bf)
        in_maps.append({
            "xt_loc": xt_loc,
            "wdq": wdq_b,
            "wuq": np.ascontiguousarray(
                wuq_x[:, HPC * hg: HPC * (hg + 1), :].reshape(Q_LORA, HPC * 256)).astype(bf),
            "wkva": wkva_x,
            "wkvb": np.ascontiguousarray(
                wkvb_eff[:, HPC * hg: HPC * (hg + 1)]).astype(bf),
            "ow": np.ascontiguousarray(ow_r[HPC * hg: HPC * (hg + 1)]).astype(bf),
            "cos_f": cosT.astype(bf),
            "sin_f": sinT.astype(bf),
            "cos_l": np.ascontiguousarray(cosT[:, s0:s0 + 512]).astype(bf),
            "sin_l": np.ascontiguousarray(sinT[:, s0:s0 + 512]).astype(bf),
            "tri": tri,
        })
    return in_maps


def _postprocess(results):
    out = np.empty((B, S, HID), dtype=np.float32)
    for b in range(B):
        acc = results[4 * b]["out_t"].astype(np.float32)
        for c in GROUPS[b][1:]:
            acc = acc + results[c]["out_t"].astype(np.float32)
        out[b] = acc.T
    return out


def kernel(**inputs):
    key = str(MM_DT)
    if key not in _CACHE:
        _CACHE[key] = build_kernel(MM_DT)
    nc = _CACHE[key]
    in_maps = _prep_inputs(**inputs)
    r = run_bass_kernel_spmd(nc, in_maps, core_ids=list(range(N_CORES)))
    return _postprocess(r.results)


# revision 24
# speedup vs baseline: 1.2032x; 1.2032x over previous
"""DeepseekV2 MLA attention prefill kernel for 8 Trainium2 NeuronCores.

Sharding: 2-way data-parallel over batch x 4-way tensor-parallel over heads
(4 heads per core).  Both the q down-projection and the compressed-KV
projection are computed on the core's S/4 local slice and exchanged with
two in-group AllGathers (ckv+kpe 590KB first, q_norm 1.5MB second); KV
decompression consumes the gathered ckv while the q gather is in flight.
Per-head up-projections, attention and the output projection are computed
locally; o_proj partial sums are written bf16 and reduced on the host
during unsharding.

Layouts: activations are feature-major ([D, S]) throughout; attention
scores are computed transposed ([s_k, s_q]) so the PV matmul needs no
transposes.  RoPE is applied via host-side permuted/sign-folded weight
columns.  Matmuls run in bf16 with fp32 PSUM accumulation.  Per-column
scale vectors (RMSNorm rstd, softmax 1/sum) are broadcast across
partitions with a K=1 ones-matmul on TensorE (GpSimd runs only the two
collectives, so nothing queues behind them).  The softmax reciprocal is
computed on VectorE (reciprocal_approx_fast), keeping ScalarE inside the
Exp activation table for the whole attention phase.  Weights are loaded
with few large DMAs; o_proj is interleaved per seq chunk with staged
writeback.
"""
import sys
sys.path.insert(0, "/opt/trn_rl_repo")

import math
import numpy as np
import ml_dtypes

import concourse.bass as bass
import concourse.tile as tile
from concourse import bacc, mybir
from concourse.bass_utils import run_bass_kernel_spmd

# ---- problem constants (hardcoded; kernel.py must be self-contained) ----
B, S, HID, H = 2, 2048, 2048, 16
Q_LORA, KV_LORA = 1536, 512
D_NOPE, D_ROPE, D_V = 128, 64, 128
D_Q = D_NOPE + D_ROPE
EPS = 1e-6
ROPE_THETA = 10000.0
N_CORES = 8
HPC = 4                      # heads per core
GROUPS = [[0, 1, 2, 3], [4, 5, 6, 7]]

F32 = mybir.dt.float32
BF16 = mybir.dt.bfloat16
F8 = mybir.dt.float8e4
MM_DT = BF16

SCALE = 1.0 / math.sqrt(D_Q)

_CACHE = {}

KC = HID // 128              # 16 contraction tiles for HID
QKC = Q_LORA // 128          # 12 contraction tiles for Q_LORA
CKC = KV_LORA // 128         # 4 contraction tiles for KV_LORA
CKV_G = KV_LORA + D_ROPE     # 576 gathered rows (ckv | roped kpe)


# ---------------------------------------------------------------- builder --
def build_kernel(mm_dt=MM_DT):
    nc = bacc.Bacc("TRN2", target_bir_lowering=False, debug=False,
                   num_devices=N_CORES)

    xt_loc = nc.dram_tensor("xt_loc", [HID, 512], mm_dt, kind="ExternalInput")
    wdq = nc.dram_tensor("wdq", [HID, Q_LORA], mm_dt, kind="ExternalInput")
    wuq = nc.dram_tensor("wuq", [Q_LORA, HPC * 256], mm_dt, kind="ExternalInput")
    wkva = nc.dram_tensor("wkva", [HID, KV_LORA + 2 * D_ROPE], mm_dt, kind="ExternalInput")
    wkvb = nc.dram_tensor("wkvb", [KV_LORA, HPC, 256], mm_dt, kind="ExternalInput")
    ow = nc.dram_tensor("ow", [HPC, D_V, HID], mm_dt, kind="ExternalInput")
    cos_f = nc.dram_tensor("cos_f", [D_ROPE, S], mm_dt, kind="ExternalInput")
    sin_f = nc.dram_tensor("sin_f", [D_ROPE, S], mm_dt, kind="ExternalInput")
    cos_l = nc.dram_tensor("cos_l", [D_ROPE, 512], mm_dt, kind="ExternalInput")
    sin_l = nc.dram_tensor("sin_l", [D_ROPE, 512], mm_dt, kind="ExternalInput")
    tri = nc.dram_tensor("tri", [128, 128], mm_dt, kind="ExternalInput")
    out_t = nc.dram_tensor("out_t", [HID, S], mm_dt, kind="ExternalOutput")

    with tile.TileContext(nc) as tc:
        import contextlib
        ctx = contextlib.ExitStack()
        with ctx:
            persist = ctx.enter_context(tc.tile_pool(name="persist", bufs=1))
            # scoped pools live on the right-side stack so they can be
            # released mid-kernel (LIFO per side): kvw after stage A, wpool
            # after stage B, then qnpool reuses the freed space for stage D
            wpool = tc.alloc_tile_pool(name="wpool", bufs=4, side="right")
            kvw = tc.alloc_tile_pool(name="kvw", bufs=1, side="right")
            spool = ctx.enter_context(tc.tile_pool(name="spool", bufs=2))
            xpool = ctx.enter_context(tc.tile_pool(name="xpool", bufs=4))
            ppool = ctx.enter_context(tc.tile_pool(name="ppool", bufs=2, space="PSUM"))
            pscore = ctx.enter_context(tc.tile_pool(name="pscore", bufs=3, space="PSUM"))
            pctx = ctx.enter_context(tc.tile_pool(name="pctx", bufs=2, space="PSUM"))
            psums = ctx.enter_context(tc.tile_pool(name="psums", bufs=1, space="PSUM"))
            dram = ctx.enter_context(tc.tile_pool(name="dram", bufs=1, space="DRAM"))

            # ---- first-need-order DMA loads -------------------------------
            # stage A gate: x local slice + kv_a weights + local rope tables
            # (interleaved piece-wise so the first matmuls start early; all
            # non-gating loads are emitted after the stage that precedes
            # their first use to keep HBM bandwidth on the critical path)
            xl_sb = []
            wkva_sb = []
            for j in range(4):
                t = xpool.tile([128, 4, 512], mm_dt, tag="xl")
                nc.sync.dma_start(
                    out=t,
                    in_=xt_loc.ap()[j * 512:(j + 1) * 512, :]
                    .rearrange("(kc p) n -> p kc n", p=128))
                xl_sb.append(t)
                w = kvw.tile([128, 4, 640], mm_dt, tag="wkva%d" % j,
                             name="wkva_sb")
                nc.sync.dma_start(
                    out=w,
                    in_=wkva.ap()[j * 512:(j + 1) * 512, :]
                    .rearrange("(kc p) c -> p kc c", p=128))
                wkva_sb.append(w)
            cosl_sb = persist.tile([D_ROPE, 512], mm_dt, tag="cosl")
            sinl_sb = persist.tile([D_ROPE, 512], mm_dt, tag="sinl")
            nc.sync.dma_start(out=cosl_sb, in_=cos_l.ap())
            nc.sync.dma_start(out=sinl_sb, in_=sin_l.ap())

            # stage B gate: q down-proj weights (first group)
            def load_wdq(mg):
                out = []
                for j in range(4):
                    t = wpool.tile([128, 4, 512], mm_dt, tag="wdq")
                    nc.sync.dma_start(
                        out=t,
                        in_=wdq.ap()[j * 512:(j + 1) * 512,
                                     mg * 512:(mg + 1) * 512]
                        .rearrange("(kc p) n -> p kc n", p=128))
                    out.append(t)
                return out

            wdq_sb = {0: load_wdq(0)}

            # constants
            ones_sb = persist.tile([128, 1], mm_dt, tag="ones")
            nc.vector.memset(ones_sb, 1.0)
            ones_row = persist.tile([1, 128], mm_dt, tag="ones_row")
            nc.vector.memset(ones_row, 1.0)
            eps_sb = persist.tile([1, 1], F32, tag="eps")
            nc.vector.memset(eps_sb, EPS)

            # gather buffers (DRAM); q travels in fp8 to halve gather time
            g_in_ckv = dram.tile([CKV_G, 512], mm_dt, name="g_in_ckv")
            g_out_ckv = dram.tile([4 * CKV_G, 512], mm_dt, name="g_out_ckv")
            g_in_q = dram.tile([Q_LORA, 512], F8, name="g_in_q")
            g_out_q = dram.tile([4 * Q_LORA, 512], F8, name="g_out_q")

            # column-broadcast helper: [1,512] f32 -> [128,512] f32 (PSUM)
            # via a K=1 ones-matmul on TensorE (keeps GpSimd free).
            def col_broadcast(vec_f32, pool, tag):
                vb = spool.tile([1, 512], mm_dt, tag="vecbf", bufs=2)
                nc.scalar.copy(vb, vec_f32)
                bc = pool.tile([128, 512], F32, tag=tag, name="bc")
                nc.tensor.matmul(bc, ones_row, vb, start=True, stop=True,
                                 skip_group_check=True)
                return bc

            # ---- stage A: local-slice compressed KV + rope + RMSNorm ------
            ckv_loc = spool.tile([128, CKC, 512], mm_dt, tag="ckv", bufs=1)
            kpe_loc = spool.tile([D_ROPE, 512], mm_dt, tag="kpe_loc", bufs=1)
            ssq_kv = psums.tile([1, 512], F32, tag="p_sum", name="ssq_kv")
            accs = [ppool.tile([128, 512], F32, tag="p_a", name="acc_kv")
                    if j < 2 else
                    pscore.tile([128, 512], F32, tag="p_sc", name="acc_kv2")
                    for j in range(5)]
            for k in range(KC):
                for j in range(5):
                    nc.tensor.matmul(
                        accs[j], wkva_sb[k // 4][:, k % 4, j * 128:(j + 1) * 128],
                        xl_sb[k // 4][:, k % 4, :],
                        start=(k == 0), stop=(k == KC - 1))
            for j in range(CKC):
                nc.vector.tensor_copy(ckv_loc[:, j, :], accs[j])
                sq = spool.tile([128, 512], mm_dt, tag="sq", bufs=1)
                nc.vector.tensor_tensor(sq, ckv_loc[:, j, :], ckv_loc[:, j, :],
                                        mybir.AluOpType.mult)
                nc.tensor.matmul(ssq_kv, ones_sb, sq,
                                 start=(j == 0), stop=(j == CKC - 1),
                                 skip_group_check=True)
            # rope chunk [E(64) | R(64)] -> kpe_loc
            t0 = spool.tile([D_ROPE, 512], F32, tag="ropet0", bufs=1)
            t1 = spool.tile([D_ROPE, 512], F32, tag="ropet1", bufs=1)
            nc.vector.tensor_tensor(t0, accs[4][0:D_ROPE, :], cosl_sb,
                                    mybir.AluOpType.mult)
            nc.vector.tensor_tensor(t1, accs[4][D_ROPE:2 * D_ROPE, :], sinl_sb,
                                    mybir.AluOpType.mult)
            nc.vector.tensor_tensor(kpe_loc, t0, t1, mybir.AluOpType.add)
            # rstd = exp(-0.5 ln(ms+eps)); broadcast on TensorE
            ls = spool.tile([1, 512], F32, tag="lsum", bufs=1)
            nc.scalar.activation(out=ls, in_=ssq_kv,
                                 func=mybir.ActivationFunctionType.Ln,
                                 bias=eps_sb, scale=1.0 / KV_LORA)
            rstd = spool.tile([1, 512], F32, tag="rstd", bufs=1)
            nc.scalar.activation(out=rstd, in_=ls, scale=-0.5,
                                 func=mybir.ActivationFunctionType.Exp)
            rstd_bc = col_broadcast(rstd, ppool, "p_a")
            for j in range(CKC):
                nc.vector.tensor_tensor(ckv_loc[:, j, :], ckv_loc[:, j, :],
                                        rstd_bc, mybir.AluOpType.mult)
            nc.sync.dma_start(
                out=g_in_ckv[0:KV_LORA, :].rearrange("(m p) n -> p m n", p=128),
                in_=ckv_loc)
            nc.sync.dma_start(out=g_in_ckv[KV_LORA:CKV_G, :], in_=kpe_loc)
            nc.gpsimd.collective_compute(
                "AllGather", mybir.AluOpType.bypass,
                replica_groups=GROUPS,
                ins=[g_in_ckv.opt()], outs=[g_out_ckv.opt()])
            kvw.release()

            # later-stage persistent loads (emitted after stage A so their
            # DMA doesn't compete with the gating loads)
            wkvb_sb = persist.tile([128, CKC, HPC, 256], mm_dt, tag="wkvb")
            nc.sync.dma_start(out=wkvb_sb,
                              in_=wkvb.ap().rearrange("(kc p) h c -> p kc h c", p=128))
            cosf_sb = persist.tile([D_ROPE, 4, 512], mm_dt, tag="cosf")
            sinf_sb = persist.tile([D_ROPE, 4, 512], mm_dt, tag="sinf")
            nc.sync.dma_start(out=cosf_sb, in_=cos_f.ap().rearrange("d (c n) -> d c n", c=4))
            nc.sync.dma_start(out=sinf_sb, in_=sin_f.ap().rearrange("d (c n) -> d c n", c=4))
            tri_sb = persist.tile([128, 128], mm_dt, tag="tri")
            nc.sync.dma_start(out=tri_sb, in_=tri.ap())
            ow_sb = persist.tile([D_V, HPC, HID], mm_dt, tag="ow")
            nc.sync.dma_start(out=ow_sb, in_=ow.ap().rearrange("h p c -> p h c"))

            # ---- stage B: q down-proj + RMSNorm on the local S chunk ------
            qnorm_own = spool.tile([128, QKC, 512], mm_dt, tag="qnorm_own", bufs=1)
            ssq_q = psums.tile([1, 512], F32, tag="p_sum", name="ssq_q")
            for mg in range(3):
                if mg not in wdq_sb:
                    wdq_sb[mg] = load_wdq(mg)
                wts = wdq_sb[mg]
                accs = [ppool.tile([128, 512], F32, tag="p_a", name="acc_q")
                        if j < 2 else
                        pscore.tile([128, 512], F32, tag="p_sc", name="acc_q2")
                        for j in range(4)]
                for k in range(KC):
                    for j in range(4):
                        nc.tensor.matmul(
                            accs[j], wts[k // 4][:, k % 4, j * 128:(j + 1) * 128],
                            xl_sb[k // 4][:, k % 4, :],
                            start=(k == 0), stop=(k == KC - 1))
                for j in range(4):
                    m = mg * 4 + j
                    nc.vector.tensor_copy(qnorm_own[:, m, :], accs[j])
                    sq = spool.tile([128, 512], mm_dt, tag="sq", bufs=1)
                    nc.vector.tensor_tensor(sq, qnorm_own[:, m, :], qnorm_own[:, m, :],
                                            mybir.AluOpType.mult)
                    nc.tensor.matmul(ssq_q, ones_sb, sq,
                                     start=(m == 0), stop=(m == QKC - 1),
                                     skip_group_check=True)
            ls2 = spool.tile([1, 512], F32, tag="lsum", bufs=1)
            nc.scalar.activation(out=ls2, in_=ssq_q,
                                 func=mybir.ActivationFunctionType.Ln,
                                 bias=eps_sb, scale=1.0 / Q_LORA)
            rstd2 = spool.tile([1, 512], F32, tag="rstd", bufs=1)
            nc.scalar.activation(out=rstd2, in_=ls2, scale=-0.5,
                                 func=mybir.ActivationFunctionType.Exp)
            rstd2_bc = col_broadcast(rstd2, ppool, "p_a")
            qfp8 = spool.tile([128, QKC, 512], F8, tag="qfp8", bufs=1)
            for m in range(QKC):
                nc.vector.tensor_tensor(qfp8[:, m, :], qnorm_own[:, m, :],
                                        rstd2_bc, mybir.AluOpType.mult)
            nc.sync.dma_start(
                out=g_in_q.rearrange("(m p) n -> p m n", p=128),
                in_=qfp8)
            nc.gpsimd.collective_compute(
                "AllGather", mybir.AluOpType.bypass,
                replica_groups=GROUPS,
                ins=[g_in_q.opt()], outs=[g_out_q.opt()])
            wpool.release()
            qnpool = tc.alloc_tile_pool(name="qnpool", bufs=6, side="right")

            # ---- stage C: KV decompression from gathered ckv --------------
            kpe_sb = persist.tile([D_ROPE, 4, 512], mm_dt, tag="kpe")
            kn_sb = persist.tile([D_NOPE, HPC, 4, 512], mm_dt, tag="kn")
            v_sb = persist.tile([128, S // 128, HPC * D_V], mm_dt, tag="v")

            for nch in range(4):
                ckv_g = xpool.tile([128, CKC, 512], mm_dt, tag="xl")
                nc.sync.dma_start(
                    out=ckv_g,
                    in_=g_out_ckv[CKV_G * nch:CKV_G * nch + KV_LORA, :]
                    .rearrange("(m p) n -> p m n", p=128))
                nc.sync.dma_start(
                    out=kpe_sb[:, nch, :],
                    in_=g_out_ckv[CKV_G * nch + KV_LORA:CKV_G * (nch + 1), :])
                for h in range(HPC):
                    acc = ppool.tile([128, 512], F32, tag="p_a", name="acc_kn")
                    for k in range(CKC):
                        nc.tensor.matmul(acc, wkvb_sb[:, k, h, 0:128],
                                         ckv_g[:, k, :],
                                         start=(k == 0), stop=(k == CKC - 1))
                    nc.scalar.copy(kn_sb[:, h, nch, :], acc)
                for st in range(4):
                    skt = nch * 4 + st
                    acc = ppool.tile([128, 512], F32, tag="p_a", name="acc_v")
                    for k in range(CKC):
                        nc.tensor.matmul(
                            acc, ckv_g[:, k, st * 128:(st + 1) * 128],
                            wkvb_sb[:, k, :, 128:256],
                            start=(k == 0), stop=(k == CKC - 1))
                    nc.scalar.copy(v_sb[:, skt, :], acc)

            # ---- stage D: per-seq-chunk q up-proj + attn + o_proj ---------
            for sqc in range(4):
                # stream this chunk's q_norm (post-gather) in 3 thirds
                qn_src = []
                for t in range(3):
                    qf = qnpool.tile([128, 4, 512], F8, tag="qn")
                    nc.sync.dma_start(
                        out=qf,
                        in_=g_out_q[Q_LORA * sqc + 512 * t:
                                         Q_LORA * sqc + 512 * (t + 1), :]
                        .rearrange("(m p) n -> p m n", p=128))
                    qn_src.append(qf)

                qn_t = {}
                qpe_t = {}
                for g2 in range(HPC):   # one head (nope + rope chunk) per pass
                    wuq_s = spool.tile([128, QKC, 256], mm_dt, tag="wuq_s", bufs=2)
                    nc.sync.dma_start(
                        out=wuq_s,
                        in_=wuq.ap()[:, g2 * 256:(g2 + 1) * 256]
                        .rearrange("(kc p) c -> p kc c", p=128))
                    accs = [ppool.tile([128, 512], F32, tag="p_a", name="acc_qup")
                            for _ in range(2)]
                    for k in range(QKC):
                        for j in range(2):
                            nc.tensor.matmul(
                                accs[j],
                                wuq_s[:, k, j * 128:(j + 1) * 128],
                                qn_src[k // 4][:, k % 4, :],
                                start=(k == 0), stop=(k == QKC - 1))
                    h = g2
                    qt = spool.tile([D_NOPE, 512], mm_dt, tag="qn_h%d" % h, bufs=1)
                    nc.scalar.copy(qt, accs[0])
                    qn_t[h] = qt
                    t0 = spool.tile([D_ROPE, 512], F32, tag="ropet0", bufs=1)
                    t1 = spool.tile([D_ROPE, 512], F32, tag="ropet1", bufs=1)
                    nc.vector.tensor_tensor(t0, accs[1][0:D_ROPE, :],
                                            cosf_sb[:, sqc, :], mybir.AluOpType.mult)
                    nc.vector.tensor_tensor(t1, accs[1][D_ROPE:2 * D_ROPE, :],
                                            sinf_sb[:, sqc, :], mybir.AluOpType.mult)
                    qpt = spool.tile([D_ROPE, 512], mm_dt, tag="qpe_h%d" % h, bufs=1)
                    nc.vector.tensor_tensor(qpt, t0, t1, mybir.AluOpType.add)
                    qpe_t[h] = qpt

                n_skt = 4 * (sqc + 1)
                ctx_sb = spool.tile([D_V, HPC, 512], mm_dt, tag="ctx", bufs=1)
                fin_pend = None   # (h, sum_acc, ctx_acc): finalize 1 head behind

                def finalize(fh, fsum, fctx):
                    # 1/sum on VectorE; broadcast on TensorE; scale on VectorE
                    sf = spool.tile([1, 512], F32, tag="sumf", bufs=2)
                    nc.scalar.copy(sf, fsum)
                    rc = spool.tile([1, 512], F32, tag="recip", bufs=2)
                    nc.vector.reciprocal_approx_fast(out=rc, in_=sf)
                    rc_bc = col_broadcast(rc, ppool, "p_a")
                    # DVE may read only one PSUM operand: stage bcast in SBUF
                    rc_sb = spool.tile([128, 512], mm_dt, tag="rc_sb", bufs=2)
                    nc.scalar.copy(rc_sb, rc_bc)
                    nc.vector.tensor_tensor(ctx_sb[:, fh, :], fctx, rc_sb,
                                            mybir.AluOpType.mult)

                for h in range(HPC):
                    sum_acc = psums.tile([1, 512], F32, tag="p_sum", name="sum_acc")
                    ctx_acc = pctx.tile([D_V, 512], F32, tag="p_ctx")
                    pending = []   # pipeline: exp tiles awaiting sums/PV
                    for skt in range(n_skt):
                        # diagonal chunk dd: columns < 128*dd are fully masked
                        # — compute only the causal column slice
                        dd = skt - 4 * sqc
                        c0 = 128 * dd if dd > 0 else 0
                        sc = pscore.tile([128, 512], F32, tag="p_sc", name="sc")
                        nc.tensor.matmul(
                            sc[:, c0:],
                            kn_sb[:, h, skt // 4, (skt % 4) * 128:(skt % 4) * 128 + 128],
                            qn_t[h][:, c0:], start=True, stop=False,
                            skip_group_check=True)
                        nc.tensor.matmul(
                            sc[:, c0:],
                            kpe_sb[:, skt // 4, (skt % 4) * 128:(skt % 4) * 128 + 128],
                            qpe_t[h][:, c0:], start=False, stop=True,
                            skip_group_check=True)
                        ex = spool.tile([128, 512], mm_dt, tag="exp%d" % (skt % 4), bufs=1)
                        nc.scalar.activation(out=ex[:, c0:], in_=sc[:, c0:],
                                             func=mybir.ActivationFunctionType.Exp,
                                             scale=SCALE)
                        if dd >= 0:
                            # only the first 128 columns of the slice touch the
                            # causal boundary — mask just that triangle
                            nc.vector.tensor_tensor(ex[:, c0:c0 + 128],
                                                    ex[:, c0:c0 + 128],
                                                    tri_sb, mybir.AluOpType.mult)
                        pending.append((ex, skt, c0))
                        if len(pending) > 3:
                            pex, pskt, pc0 = pending.pop(0)
                            nc.tensor.matmul(sum_acc[:, pc0:], ones_sb, pex[:, pc0:],
                                             start=(pskt == 0), stop=False,
                                             skip_group_check=True)
                            nc.tensor.matmul(ctx_acc[:, pc0:],
                                             v_sb[:, pskt, h * D_V:(h + 1) * D_V],
                                             pex[:, pc0:], start=(pskt == 0), stop=False,
                                             skip_group_check=True)
                        if skt == 1 and fin_pend is not None:
                            finalize(*fin_pend)
                            fin_pend = None
                    while pending:
                        pex, pskt, pc0 = pending.pop(0)
                        last = not pending
                        nc.tensor.matmul(sum_acc[:, pc0:], ones_sb, pex[:, pc0:],
                                         start=(pskt == 0), stop=last,
                                         skip_group_check=True)
                        nc.tensor.matmul(ctx_acc[:, pc0:],
                                         v_sb[:, pskt, h * D_V:(h + 1) * D_V],
                                         pex[:, pc0:], start=(pskt == 0), stop=last,
                                         skip_group_check=True)
                    fin_pend = (h, sum_acc, ctx_acc)
                finalize(*fin_pend)
                fin_pend = None

                # o_proj for this seq chunk (partial sums over local heads)
                for og in range(4):
                    ostage = spool.tile([128, 4, 512], mm_dt, tag="ostage", bufs=2)
                    for hc in range(4):
                        hidc = og * 4 + hc
                        acc = pctx.tile([128, 512], F32, tag="p_ctx", name="acc_o")
                        for h in range(HPC):
                            nc.tensor.matmul(acc, ow_sb[:, h, hidc * 128:(hidc + 1) * 128],
                                             ctx_sb[:, h, :],
                                             start=(h == 0), stop=(h == HPC - 1))
                        nc.scalar.copy(ostage[:, hc, :], acc)
                    nc.sync.dma_start(
                        out=out_t.ap()[og * 512:(og + 1) * 512,
                                       sqc * 512:(sqc + 1) * 512]
                        .rearrange("(hc p) n -> p hc n", p=128),
                        in_=ostage)
            qnpool.release()

    nc.compile()
    return nc


# ------------------------------------------------------------- host side --
def _rope_tables():
    inv_freq = 1.0 / (ROPE_THETA ** (np.arange(0, D_ROPE, 2, dtype=np.float64) / D_ROPE))
    t = np.arange(S, dtype=np.float64)
    freqs = np.outer(t, inv_freq)                    # [S, 32]
    emb = np.concatenate([freqs, freqs], axis=-1)    # [S, 64]
    return (np.cos(emb).astype(np.float32).T.copy(),
            np.sin(emb).astype(np.float32).T.copy())  # [64, S]


_E_PERM = np.concatenate([np.arange(0, D_ROPE, 2), np.arange(1, D_ROPE, 2)])


def _rope_expand(Wpe):
    """[n, 64] rope weight cols -> [n, 128]: [even/odd-reordered | rot-half signed]."""
    Y = Wpe[:, _E_PERM]
    R = np.concatenate([-Y[:, D_ROPE // 2:], Y[:, :D_ROPE // 2]], axis=1)
    return np.concatenate([Y, R], axis=1)


def _prep_inputs(hidden_states, w_dq, q_a_ln_w, w_uq, kv_a_w, kv_a_ln_w, kv_b_w, o_w):
    bf = ml_dtypes.bfloat16
    cosT, sinT = _rope_tables()

    wuq_eff = (np.asarray(q_a_ln_w)[:, None] * np.asarray(w_uq)).reshape(Q_LORA, H, D_Q)
    head_blocks = []
    for h in range(H):
        head_blocks.append(np.concatenate(
            [wuq_eff[:, h, :D_NOPE], _rope_expand(wuq_eff[:, h, D_NOPE:])], axis=1))
    wuq_x = np.stack(head_blocks, axis=1)            # [1536, 16, 256]

    kv_a = np.asarray(kv_a_w)
    wkva_x = np.concatenate([kv_a[:, :KV_LORA], _rope_expand(kv_a[:, KV_LORA:])],
                            axis=1).astype(bf)       # [2048, 640]
    wkvb_eff = (np.asarray(kv_a_ln_w)[:, None] * np.asarray(kv_b_w)).reshape(KV_LORA, H, 256)
    ow_r = np.asarray(o_w).reshape(H, D_V, HID)

    tri = (np.arange(128)[None, :] >= np.arange(128)[:, None]).astype(bf)

    wdq_b = np.asarray(w_dq).astype(bf)
    hs = np.asarray(hidden_states)

    in_maps = []
    for c in range(N_CORES):
        b, hg = c // 4, c % 4
        s0 = 512 * hg
        xt_loc = np.ascontiguousarray(hs[b].T[:, s0:s0 + 512]).astype(bf)
        in_maps.append({
            "xt_loc": xt_loc,
            "wdq": wdq_b,
            "wuq": np.ascontiguousarray(
                wuq_x[:, HPC * hg: HPC * (hg + 1), :].reshape(Q_LORA, HPC * 256)).astype(bf),
            "wkva": wkva_x,
            "wkvb": np.ascontiguousarray(
                wkvb_eff[:, HPC * hg: HPC * (hg + 1)]).astype(bf),
            "ow": np.ascontiguousarray(ow_r[HPC * hg: HPC * (hg + 1)]).astype(bf),
            "cos_f": cosT.astype(bf),
            "sin_f": sinT.astype(bf),
            "cos_l": np.ascontiguousarray(cosT[:, s0:s0 + 512]).astype(bf),
            "sin_l": np.ascontiguousarray(sinT[:, s0:s0 + 512]).astype(bf),
            "tri": tri,
        })
    return in_maps


def _postprocess(results):
    out = np.empty((B, S, HID), dtype=np.float32)
    for b in range(B):
        acc = results[4 * b]["out_t"].astype(np.float32)
        for c in GROUPS[b][1:]:
            acc = acc + results[c]["out_t"].astype(np.float32)
        out[b] = acc.T
    return out


def kernel(**inputs):
    key = str(MM_DT)
    if key not in _CACHE:
        _CACHE[key] = build_kernel(MM_DT)
    nc = _CACHE[key]
    in_maps = _prep_inputs(**inputs)
    r = run_bass_kernel_spmd(nc, in_maps, core_ids=list(range(N_CORES)))
    return _postprocess(r.results)


# revision 42
# speedup vs baseline: 1.4658x; 1.2183x over previous
"""DeepseekV2 MLA attention prefill kernel for 8 Trainium2 NeuronCores.

Sharding: 2-way data-parallel over batch x 4-way tensor-parallel over heads
(4 heads per core).  Both the q down-projection and the compressed-KV
projection are computed on the core's S/4 local slice and exchanged with
two in-group AllGathers (ckv+kpe 590KB first, q_norm 1.5MB second); KV
decompression consumes the gathered ckv while the q gather is in flight.
Per-head up-projections, attention and the output projection are computed
locally; o_proj partial sums are written bf16 and reduced on the host
during unsharding.

Layouts: activations are feature-major ([D, S]) throughout; attention
scores are computed transposed ([s_k, s_q]) so the PV matmul needs no
transposes.  RoPE is applied via host-side permuted/sign-folded weight
columns.  Matmuls run in bf16 with fp32 PSUM accumulation.  Per-column
scale vectors (RMSNorm rstd, softmax 1/sum) are broadcast across
partitions with a K=1 ones-matmul on TensorE (GpSimd runs only the two
collectives, so nothing queues behind them).  The softmax reciprocal is
computed on VectorE (reciprocal_approx_fast), keeping ScalarE inside the
Exp activation table for the whole attention phase.  Weights are loaded
with few large DMAs; o_proj is interleaved per seq chunk with staged
writeback.
"""
import sys
sys.path.insert(0, "/opt/trn_rl_repo")

import math
import numpy as np
import ml_dtypes

import concourse.bass as bass
import concourse.tile as tile
from concourse import bacc, mybir
from concourse.bass_utils import run_bass_kernel_spmd

# ---- problem constants (hardcoded; kernel.py must be self-contained) ----
B, S, HID, H = 2, 2048, 2048, 16
Q_LORA, KV_LORA = 1536, 512
D_NOPE, D_ROPE, D_V = 128, 64, 128
D_Q = D_NOPE + D_ROPE
EPS = 1e-6
ROPE_THETA = 10000.0
N_CORES = 8
HPC = 4                      # heads per core
GROUPS = [[0, 1, 2, 3], [4, 5, 6, 7]]

F32 = mybir.dt.float32
BF16 = mybir.dt.bfloat16
F8 = mybir.dt.float8e4
MM_DT = BF16

SCALE = 1.0 / math.sqrt(D_Q)
WUQ_GAIN = 32.0              # host pre-scales fp8 wuq out of the subnormal
EXP_SCALE = SCALE / WUQ_GAIN  # range; folded back here (scores carry x32)

_CACHE = {}

KC = HID // 128              # 16 contraction tiles for HID
QKC = Q_LORA // 128          # 12 contraction tiles for Q_LORA
CKC = KV_LORA // 128         # 4 contraction tiles for KV_LORA
CKV_G = KV_LORA + D_ROPE     # 576 gathered rows (ckv | roped kpe)


# ---------------------------------------------------------------- builder --
def build_kernel(mm_dt=MM_DT):
    nc = bacc.Bacc("TRN2", target_bir_lowering=False, debug=False,
                   num_devices=N_CORES)

    xt_loc = nc.dram_tensor("xt_loc", [HID, 512], mm_dt, kind="ExternalInput")
    wdq = nc.dram_tensor("wdq", [HID, Q_LORA], mm_dt, kind="ExternalInput")
    wuq = nc.dram_tensor("wuq", [Q_LORA, HPC * 256], F8, kind="ExternalInput")
    wkva = nc.dram_tensor("wkva", [HID, KV_LORA + 2 * D_ROPE], mm_dt, kind="ExternalInput")
    wkvb = nc.dram_tensor("wkvb", [KV_LORA, HPC, 256], mm_dt, kind="ExternalInput")
    ow = nc.dram_tensor("ow", [HPC, D_V, HID], mm_dt, kind="ExternalInput")
    cos_f = nc.dram_tensor("cos_f", [D_ROPE, S], mm_dt, kind="ExternalInput")
    sin_f = nc.dram_tensor("sin_f", [D_ROPE, S], mm_dt, kind="ExternalInput")
    cos_l = nc.dram_tensor("cos_l", [D_ROPE, 512], mm_dt, kind="ExternalInput")
    sin_l = nc.dram_tensor("sin_l", [D_ROPE, 512], mm_dt, kind="ExternalInput")
    tri = nc.dram_tensor("tri", [128, 128], mm_dt, kind="ExternalInput")
    out_t = nc.dram_tensor("out_t", [HID, S], mm_dt, kind="ExternalOutput")

    with tile.TileContext(nc) as tc:
        import contextlib
        ctx = contextlib.ExitStack()
        with ctx:
            persist = ctx.enter_context(tc.tile_pool(name="persist", bufs=1))
            # scoped pools live on the right-side stack so they can be
            # released mid-kernel (LIFO per side): kvw after stage A, wpool
            # after stage B, then qnpool reuses the freed space for stage D
            wpool = tc.alloc_tile_pool(name="wpool", bufs=4, side="right")
            kvw = tc.alloc_tile_pool(name="kvw", bufs=1, side="right")
            spool = ctx.enter_context(tc.tile_pool(name="spool", bufs=2))
            xpool = ctx.enter_context(tc.tile_pool(name="xpool", bufs=4))
            ppool = ctx.enter_context(tc.tile_pool(name="ppool", bufs=2, space="PSUM"))
            pscore = ctx.enter_context(tc.tile_pool(name="pscore", bufs=3, space="PSUM"))
            pctx = ctx.enter_context(tc.tile_pool(name="pctx", bufs=2, space="PSUM"))
            psums = ctx.enter_context(tc.tile_pool(name="psums", bufs=1, space="PSUM"))
            dram = ctx.enter_context(tc.tile_pool(name="dram", bufs=1, space="DRAM"))

            # ---- first-need-order DMA loads -------------------------------
            # stage A gate: x local slice + kv_a weights + local rope tables
            # (interleaved piece-wise so the first matmuls start early; all
            # non-gating loads are emitted after the stage that precedes
            # their first use to keep HBM bandwidth on the critical path)
            xl_sb = []
            wkva_sb = []
            for j in range(4):
                t = xpool.tile([128, 4, 512], mm_dt, tag="xl")
                nc.sync.dma_start(
                    out=t,
                    in_=xt_loc.ap()[j * 512:(j + 1) * 512, :]
                    .rearrange("(kc p) n -> p kc n", p=128))
                xl_sb.append(t)
                w = kvw.tile([128, 4, 640], mm_dt, tag="wkva%d" % j,
                             name="wkva_sb")
                nc.sync.dma_start(
                    out=w,
                    in_=wkva.ap()[j * 512:(j + 1) * 512, :]
                    .rearrange("(kc p) c -> p kc c", p=128))
                wkva_sb.append(w)
            cosl_sb = persist.tile([D_ROPE, 512], mm_dt, tag="cosl")
            sinl_sb = persist.tile([D_ROPE, 512], mm_dt, tag="sinl")
            nc.sync.dma_start(out=cosl_sb, in_=cos_l.ap())
            nc.sync.dma_start(out=sinl_sb, in_=sin_l.ap())

            # stage B gate: q down-proj weights (first group)
            def load_wdq(mg):
                out = []
                for j in range(4):
                    t = wpool.tile([128, 4, 512], mm_dt, tag="wdq")
                    nc.sync.dma_start(
                        out=t,
                        in_=wdq.ap()[j * 512:(j + 1) * 512,
                                     mg * 512:(mg + 1) * 512]
                        .rearrange("(kc p) n -> p kc n", p=128))
                    out.append(t)
                return out

            wdq_sb = {0: load_wdq(0)}

            # constants
            ones_sb = persist.tile([128, 1], mm_dt, tag="ones")
            nc.vector.memset(ones_sb, 1.0)
            ones_row = persist.tile([1, 128], mm_dt, tag="ones_row")
            nc.vector.memset(ones_row, 1.0)
            eps_sb = persist.tile([1, 1], F32, tag="eps")
            nc.vector.memset(eps_sb, EPS)

            # gather buffers (DRAM); q travels in fp8 to halve gather time
            g_in_ckv = dram.tile([CKV_G, 512], mm_dt, name="g_in_ckv")
            g_out_ckv = dram.tile([4 * CKV_G, 512], mm_dt, name="g_out_ckv")
            g_in_q = dram.tile([Q_LORA, 512], F8, name="g_in_q")
            g_out_q = dram.tile([4 * Q_LORA, 512], F8, name="g_out_q")

            # column-broadcast helper: [1,512] f32 -> [128,512] f32 (PSUM)
            # via a K=1 ones-matmul on TensorE (keeps GpSimd free).
            def col_broadcast(vec_f32, pool, tag):
                vb = spool.tile([1, 512], mm_dt, tag="vecbf", bufs=2)
                nc.scalar.copy(vb, vec_f32)
                bc = pool.tile([128, 512], F32, tag=tag, name="bc")
                nc.tensor.matmul(bc, ones_row, vb, start=True, stop=True,
                                 skip_group_check=True)
                return bc

            # ---- stage A: local-slice compressed KV + rope + RMSNorm ------
            ckv_loc = spool.tile([128, CKC, 512], mm_dt, tag="ckv", bufs=1)
            kpe_loc = spool.tile([D_ROPE, 512], mm_dt, tag="kpe_loc", bufs=1)
            ssq_kv = psums.tile([1, 512], F32, tag="p_sum", name="ssq_kv")
            accs = [ppool.tile([128, 512], F32, tag="p_a", name="acc_kv")
                    if j < 2 else
                    pscore.tile([128, 512], F32, tag="p_sc", name="acc_kv2")
                    for j in range(5)]
            for k in range(KC):
                for j in range(5):
                    nc.tensor.matmul(
                        accs[j], wkva_sb[k // 4][:, k % 4, j * 128:(j + 1) * 128],
                        xl_sb[k // 4][:, k % 4, :],
                        start=(k == 0), stop=(k == KC - 1))
            for j in range(CKC):
                nc.vector.tensor_copy(ckv_loc[:, j, :], accs[j])
                sq = spool.tile([128, 512], mm_dt, tag="sq", bufs=1)
                nc.vector.tensor_tensor(sq, ckv_loc[:, j, :], ckv_loc[:, j, :],
                                        mybir.AluOpType.mult)
                nc.tensor.matmul(ssq_kv, ones_sb, sq,
                                 start=(j == 0), stop=(j == CKC - 1),
                                 skip_group_check=True)
            # rope chunk [E(64) | R(64)] -> kpe_loc
            t0 = spool.tile([D_ROPE, 512], F32, tag="ropet0", bufs=1)
            t1 = spool.tile([D_ROPE, 512], F32, tag="ropet1", bufs=1)
            nc.vector.tensor_tensor(t0, accs[4][0:D_ROPE, :], cosl_sb,
                                    mybir.AluOpType.mult)
            nc.vector.tensor_tensor(t1, accs[4][D_ROPE:2 * D_ROPE, :], sinl_sb,
                                    mybir.AluOpType.mult)
            nc.vector.tensor_tensor(kpe_loc, t0, t1, mybir.AluOpType.add)
            # rstd = exp(-0.5 ln(ms+eps)); broadcast on TensorE
            ls = spool.tile([1, 512], F32, tag="lsum", bufs=1)
            nc.scalar.activation(out=ls, in_=ssq_kv,
                                 func=mybir.ActivationFunctionType.Ln,
                                 bias=eps_sb, scale=1.0 / KV_LORA)
            rstd = spool.tile([1, 512], F32, tag="rstd", bufs=1)
            nc.scalar.activation(out=rstd, in_=ls, scale=-0.5,
                                 func=mybir.ActivationFunctionType.Exp)
            rstd_bc = col_broadcast(rstd, ppool, "p_a")
            for j in range(CKC):
                nc.vector.tensor_tensor(ckv_loc[:, j, :], ckv_loc[:, j, :],
                                        rstd_bc, mybir.AluOpType.mult)
            nc.sync.dma_start(
                out=g_in_ckv[0:KV_LORA, :].rearrange("(m p) n -> p m n", p=128),
                in_=ckv_loc)
            nc.sync.dma_start(out=g_in_ckv[KV_LORA:CKV_G, :], in_=kpe_loc)
            nc.gpsimd.collective_compute(
                "AllGather", mybir.AluOpType.bypass,
                replica_groups=GROUPS,
                ins=[g_in_ckv.opt()], outs=[g_out_ckv.opt()])
            kvw.release()

            # later-stage persistent loads (emitted after stage A so their
            # DMA doesn't compete with the gating loads)
            wkvb_sb = persist.tile([128, CKC, HPC, 256], mm_dt, tag="wkvb")
            nc.sync.dma_start(out=wkvb_sb,
                              in_=wkvb.ap().rearrange("(kc p) h c -> p kc h c", p=128))
            cosf_sb = persist.tile([D_ROPE, 4, 512], mm_dt, tag="cosf")
            sinf_sb = persist.tile([D_ROPE, 4, 512], mm_dt, tag="sinf")
            nc.sync.dma_start(out=cosf_sb, in_=cos_f.ap().rearrange("d (c n) -> d c n", c=4))
            nc.sync.dma_start(out=sinf_sb, in_=sin_f.ap().rearrange("d (c n) -> d c n", c=4))
            tri_sb = persist.tile([128, 128], mm_dt, tag="tri")
            nc.sync.dma_start(out=tri_sb, in_=tri.ap())
            ow_sb = persist.tile([D_V, HPC, HID], mm_dt, tag="ow")
            nc.sync.dma_start(out=ow_sb, in_=ow.ap().rearrange("h p c -> p h c"))

            # ---- stage B: q down-proj + RMSNorm on the local S chunk ------
            qnorm_own = spool.tile([128, QKC, 512], mm_dt, tag="qnorm_own", bufs=1)
            ssq_q = psums.tile([1, 512], F32, tag="p_sum", name="ssq_q")
            for mg in range(3):
                if mg not in wdq_sb:
                    wdq_sb[mg] = load_wdq(mg)
                wts = wdq_sb[mg]
                accs = [ppool.tile([128, 512], F32, tag="p_a", name="acc_q")
                        if j < 2 else
                        pscore.tile([128, 512], F32, tag="p_sc", name="acc_q2")
                        for j in range(4)]
                for k in range(KC):
                    for j in range(4):
                        nc.tensor.matmul(
                            accs[j], wts[k // 4][:, k % 4, j * 128:(j + 1) * 128],
                            xl_sb[k // 4][:, k % 4, :],
                            start=(k == 0), stop=(k == KC - 1))
                for j in range(4):
                    m = mg * 4 + j
                    nc.vector.tensor_copy(qnorm_own[:, m, :], accs[j])
                    sq = spool.tile([128, 512], mm_dt, tag="sq", bufs=1)
                    nc.vector.tensor_tensor(sq, qnorm_own[:, m, :], qnorm_own[:, m, :],
                                            mybir.AluOpType.mult)
                    nc.tensor.matmul(ssq_q, ones_sb, sq,
                                     start=(m == 0), stop=(m == QKC - 1),
                                     skip_group_check=True)
            ls2 = spool.tile([1, 512], F32, tag="lsum", bufs=1)
            nc.scalar.activation(out=ls2, in_=ssq_q,
                                 func=mybir.ActivationFunctionType.Ln,
                                 bias=eps_sb, scale=1.0 / Q_LORA)
            rstd2 = spool.tile([1, 512], F32, tag="rstd", bufs=1)
            nc.scalar.activation(out=rstd2, in_=ls2, scale=-0.5,
                                 func=mybir.ActivationFunctionType.Exp)
            rstd2_bc = col_broadcast(rstd2, ppool, "p_a")
            qfp8 = spool.tile([128, QKC, 512], F8, tag="qfp8", bufs=1)
            for m in range(QKC):
                nc.vector.tensor_tensor(qfp8[:, m, :], qnorm_own[:, m, :],
                                        rstd2_bc, mybir.AluOpType.mult)
            nc.sync.dma_start(
                out=g_in_q.rearrange("(m p) n -> p m n", p=128),
                in_=qfp8)
            nc.gpsimd.collective_compute(
                "AllGather", mybir.AluOpType.bypass,
                replica_groups=GROUPS,
                ins=[g_in_q.opt()], outs=[g_out_q.opt()])
            wpool.release()
            qnpool = tc.alloc_tile_pool(name="qnpool", bufs=6, side="right")

            # ---- stage C: KV decompression from gathered ckv --------------
            # fp8 packed layouts for DoubleRow attention:
            #   knx[:, h, skt, 0, :] = k_nope chunk;  knx[0:64, h, skt, 1, :]
            #   = shared roped kpe;  knx[64:, h, skt, 1, :] = 0 (pads the
            #   2nd contraction slot).  v stays bf16: value-path quantization
            #   hits the output linearly (no softmax damping).
            kpe_sb = persist.tile([D_ROPE, 4, 512], mm_dt, tag="kpe")
            knx = persist.tile([128, HPC, 16, 2, 128], F8, tag="knx")
            v_sb = persist.tile([128, S // 128, HPC * D_V], mm_dt, tag="v")
            nc.vector.memset(knx[64:128, :, :, 1, :], 0.0)

            ckv_gs = []
            for nch in range(4):
                ckv_g = xpool.tile([128, CKC, 512], mm_dt, tag="xl")
                nc.sync.dma_start(
                    out=ckv_g,
                    in_=g_out_ckv[CKV_G * nch:CKV_G * nch + KV_LORA, :]
                    .rearrange("(m p) n -> p m n", p=128))
                nc.sync.dma_start(
                    out=kpe_sb[:, nch, :],
                    in_=g_out_ckv[CKV_G * nch + KV_LORA:CKV_G * (nch + 1), :])
                ckv_gs.append(ckv_g)
                for h in range(HPC):
                    acc = ppool.tile([128, 512], F32, tag="p_a", name="acc_kn")
                    for k in range(CKC):
                        nc.tensor.matmul(acc, wkvb_sb[:, k, h, 0:128],
                                         ckv_g[:, k, :],
                                         start=(k == 0), stop=(k == CKC - 1))
                    nc.scalar.copy(knx[:, h, 4 * nch:4 * nch + 4, 0, :], acc)
                    nc.scalar.copy(knx[0:D_ROPE, h, 4 * nch:4 * nch + 4, 1, :],
                                   kpe_sb[:, nch, :])

            def emit_v(nch):
                for st in range(4):
                    skt = nch * 4 + st
                    acc = ppool.tile([128, 512], F32, tag="p_a", name="acc_v")
                    for k in range(CKC):
                        nc.tensor.matmul(
                            acc, ckv_gs[nch][:, k, st * 128:(st + 1) * 128],
                            wkvb_sb[:, k, :, 128:256],
                            start=(k == 0), stop=(k == CKC - 1))
                    nc.scalar.copy(v_sb[:, skt, :], acc)

            emit_v(0)
            emit_v(1)
            # (v for chunks 8-15 is emitted after sqc0's q up-projection so
            # TensorE has ready work while the q AllGather completes)

            # ---- stage D: per-seq-chunk q up-proj + attn + o_proj ---------
            # per-head fp8 q tiles [d, 2, 512]: slot0 = q_nope, slot1[0:64]
            # = roped q_pe; bottom half of slot1 zeroed once (meets the
            # zero-padded half of knx slot1)
            qx_t = []
            for h in range(HPC):
                qx = spool.tile([128, 2, 512], F8, tag="qx_h%d" % h, bufs=1,
                                name="qx")
                nc.vector.memset(qx[64:128, 1, :], 0.0)
                qx_t.append(qx)
            DR = mybir.MatmulPerfMode.DoubleRow

            for sqc in range(4):
                # stream this chunk's q_norm (post-gather) in 3 thirds
                qn_src = []
                for t in range(3):
                    qf = qnpool.tile([128, 4, 512], F8, tag="qn")
                    nc.sync.dma_start(
                        out=qf,
                        in_=g_out_q[Q_LORA * sqc + 512 * t:
                                         Q_LORA * sqc + 512 * (t + 1), :]
                        .rearrange("(m p) n -> p m n", p=128))
                    qn_src.append(qf)

                for g2 in range(HPC):   # one head (nope + rope chunk) per pass
                    wuq_s = spool.tile([128, QKC, 256], F8, tag="wuq_s", bufs=2)
                    nc.sync.dma_start(
                        out=wuq_s,
                        in_=wuq.ap()[:, g2 * 256:(g2 + 1) * 256]
                        .rearrange("(kc p) c -> p kc c", p=128))
                    accs = [ppool.tile([128, 512], F32, tag="p_a", name="acc_qup")
                            for _ in range(2)]
                    for kp in range(QKC // 2):   # fp8 DoubleRow: k-tile pairs
                        t, u = kp // 2, (kp % 2) * 2
                        for j in range(2):
                            nc.tensor.matmul(
                                accs[j],
                                wuq_s[:, 2 * kp:2 * kp + 2, j * 128:(j + 1) * 128],
                                qn_src[t][:, u:u + 2, :],
                                start=(kp == 0), stop=(kp == QKC // 2 - 1),
                                perf_mode=DR)
                    h = g2
                    nc.scalar.copy(qx_t[h][:, 0, :], accs[0])
                    t0 = spool.tile([D_ROPE, 512], F32, tag="ropet0", bufs=1)
                    t1 = spool.tile([D_ROPE, 512], F32, tag="ropet1", bufs=1)
                    nc.vector.tensor_tensor(t0, accs[1][0:D_ROPE, :],
                                            cosf_sb[:, sqc, :], mybir.AluOpType.mult)
                    nc.vector.tensor_tensor(t1, accs[1][D_ROPE:2 * D_ROPE, :],
                                            sinf_sb[:, sqc, :], mybir.AluOpType.mult)
                    nc.vector.tensor_tensor(qx_t[h][0:D_ROPE, 1, :], t0, t1,
                                            mybir.AluOpType.add)

                if sqc == 0:
                    emit_v(2)
                    emit_v(3)

                n_skt = 4 * (sqc + 1)
                ctx_sb = spool.tile([D_V, HPC, 512], mm_dt, tag="ctx", bufs=1)
                fin_pend = None   # (h, sum_acc, ctx_acc): finalize 1 head behind

                def finalize(fh, fsum, fctx):
                    # 1/sum on VectorE; broadcast on TensorE; scale on VectorE
                    sf = spool.tile([1, 512], F32, tag="sumf", bufs=2)
                    nc.scalar.copy(sf, fsum)
                    rc = spool.tile([1, 512], F32, tag="recip", bufs=2)
                    nc.vector.reciprocal_approx_fast(out=rc, in_=sf)
                    rc_bc = col_broadcast(rc, ppool, "p_a")
                    # DVE may read only one PSUM operand: stage bcast in SBUF
                    rc_sb = spool.tile([128, 512], mm_dt, tag="rc_sb", bufs=2)
                    nc.scalar.copy(rc_sb, rc_bc)
                    nc.vector.tensor_tensor(ctx_sb[:, fh, :], fctx, rc_sb,
                                            mybir.AluOpType.mult)

                for h in range(HPC):
                    sum_acc = psums.tile([1, 512], F32, tag="p_sum", name="sum_acc")
                    ctx_acc = pctx.tile([D_V, 512], F32, tag="p_ctx")
                    pending = []   # pipeline: exp tiles awaiting sums/PV
                    for skt in range(n_skt):
                        # diagonal chunk dd: columns < 128*dd are fully masked
                        # — compute only the causal column slice
                        dd = skt - 4 * sqc
                        c0 = 128 * dd if dd > 0 else 0
                        sc = pscore.tile([128, 512], F32, tag="p_sc", name="sc")
                        nc.tensor.matmul(
                            sc[:, c0:], knx[:, h, skt, :, :],
                            qx_t[h][:, :, c0:], start=True, stop=True,
                            perf_mode=DR, skip_group_check=True)
                        ex = spool.tile([128, 512], mm_dt, tag="exp%d" % (skt % 4), bufs=1)
                        nc.scalar.activation(out=ex[:, c0:], in_=sc[:, c0:],
                                             func=mybir.ActivationFunctionType.Exp,
                                             scale=EXP_SCALE)
                        if dd >= 0:
                            # only the first 128 columns of the slice touch the
                            # causal boundary — mask just that triangle
                            nc.vector.tensor_tensor(ex[:, c0:c0 + 128],
                                                    ex[:, c0:c0 + 128],
                                                    tri_sb, mybir.AluOpType.mult)
                        pending.append((ex, skt, c0))
                        if len(pending) > 3:
                            pex, pskt, pc0 = pending.pop(0)
                            nc.tensor.matmul(sum_acc[:, pc0:], ones_sb, pex[:, pc0:],
                                             start=(pskt == 0), stop=False,
                                             skip_group_check=True)
                            nc.tensor.matmul(ctx_acc[:, pc0:],
                                             v_sb[:, pskt, h * D_V:(h + 1) * D_V],
                                             pex[:, pc0:], start=(pskt == 0), stop=False,
                                             skip_group_check=True)
                        if skt == 1 and fin_pend is not None:
                            finalize(*fin_pend)
                            fin_pend = None
                    while pending:
                        pex, pskt, pc0 = pending.pop(0)
                        last = not pending
                        nc.tensor.matmul(sum_acc[:, pc0:], ones_sb, pex[:, pc0:],
                                         start=(pskt == 0), stop=last,
                                         skip_group_check=True)
                        nc.tensor.matmul(ctx_acc[:, pc0:],
                                         v_sb[:, pskt, h * D_V:(h + 1) * D_V],
                                         pex[:, pc0:], start=(pskt == 0), stop=last,
                                         skip_group_check=True)
                    fin_pend = (h, sum_acc, ctx_acc)
                finalize(*fin_pend)
                fin_pend = None

                # o_proj for this seq chunk (partial sums over local heads)
                for og in range(4):
                    ostage = spool.tile([128, 4, 512], mm_dt, tag="ostage", bufs=2)
                    for hc in range(4):
                        hidc = og * 4 + hc
                        acc = pctx.tile([128, 512], F32, tag="p_ctx", name="acc_o")
                        for h in range(HPC):
                            nc.tensor.matmul(acc, ow_sb[:, h, hidc * 128:(hidc + 1) * 128],
                                             ctx_sb[:, h, :],
                                             start=(h == 0), stop=(h == HPC - 1))
                        nc.scalar.copy(ostage[:, hc, :], acc)
                    nc.sync.dma_start(
                        out=out_t.ap()[og * 512:(og + 1) * 512,
                                       sqc * 512:(sqc + 1) * 512]
                        .rearrange("(hc p) n -> p hc n", p=128),
                        in_=ostage)
            qnpool.release()

    nc.compile()
    return nc


# ------------------------------------------------------------- host side --
def _rope_tables():
    inv_freq = 1.0 / (ROPE_THETA ** (np.arange(0, D_ROPE, 2, dtype=np.float64) / D_ROPE))
    t = np.arange(S, dtype=np.float64)
    freqs = np.outer(t, inv_freq)                    # [S, 32]
    emb = np.concatenate([freqs, freqs], axis=-1)    # [S, 64]
    return (np.cos(emb).astype(np.float32).T.copy(),
            np.sin(emb).astype(np.float32).T.copy())  # [64, S]


_E_PERM = np.concatenate([np.arange(0, D_ROPE, 2), np.arange(1, D_ROPE, 2)])


def _rope_expand(Wpe):
    """[n, 64] rope weight cols -> [n, 128]: [even/odd-reordered | rot-half signed]."""
    Y = Wpe[:, _E_PERM]
    R = np.concatenate([-Y[:, D_ROPE // 2:], Y[:, :D_ROPE // 2]], axis=1)
    return np.concatenate([Y, R], axis=1)


def _prep_inputs(hidden_states, w_dq, q_a_ln_w, w_uq, kv_a_w, kv_a_ln_w, kv_b_w, o_w):
    bf = ml_dtypes.bfloat16
    cosT, sinT = _rope_tables()

    wuq_eff = (np.asarray(q_a_ln_w)[:, None] * np.asarray(w_uq)).reshape(Q_LORA, H, D_Q)
    head_blocks = []
    for h in range(H):
        head_blocks.append(np.concatenate(
            [wuq_eff[:, h, :D_NOPE], _rope_expand(wuq_eff[:, h, D_NOPE:])], axis=1))
    wuq_x = np.stack(head_blocks, axis=1)            # [1536, 16, 256]

    kv_a = np.asarray(kv_a_w)
    wkva_x = np.concatenate([kv_a[:, :KV_LORA], _rope_expand(kv_a[:, KV_LORA:])],
                            axis=1).astype(bf)       # [2048, 640]
    wkvb_eff = (np.asarray(kv_a_ln_w)[:, None] * np.asarray(kv_b_w)).reshape(KV_LORA, H, 256)
    ow_r = np.asarray(o_w).reshape(H, D_V, HID)

    tri = (np.arange(128)[None, :] >= np.arange(128)[:, None]).astype(bf)

    wdq_b = np.asarray(w_dq).astype(bf)
    hs = np.asarray(hidden_states)

    in_maps = []
    for c in range(N_CORES):
        b, hg = c // 4, c % 4
        s0 = 512 * hg
        xt_loc = np.ascontiguousarray(hs[b].T[:, s0:s0 + 512]).astype(bf)
        in_maps.append({
            "xt_loc": xt_loc,
            "wdq": wdq_b,
            "wuq": np.ascontiguousarray(
                WUQ_GAIN * wuq_x[:, HPC * hg: HPC * (hg + 1), :]
                .reshape(Q_LORA, HPC * 256)).astype(ml_dtypes.float8_e4m3fn),
            "wkva": wkva_x,
            "wkvb": np.ascontiguousarray(
                wkvb_eff[:, HPC * hg: HPC * (hg + 1)]).astype(bf),
            "ow": np.ascontiguousarray(ow_r[HPC * hg: HPC * (hg + 1)]).astype(bf),
            "cos_f": cosT.astype(bf),
            "sin_f": sinT.astype(bf),
            "cos_l": np.ascontiguousarray(cosT[:, s0:s0 + 512]).astype(bf),
            "sin_l": np.ascontiguousarray(sinT[:, s0:s0 + 512]).astype(bf),
            "tri": tri,
        })
    return in_maps


def _postprocess(results):
    out = np.empty((B, S, HID), dtype=np.float32)
    for b in range(B):
        acc = results[4 * b]["out_t"].astype(np.float32)
        for c in GROUPS[b][1:]:
            acc = acc + results[c]["out_t"].astype(np.float32)
        out[b] = acc.T
    return out


def kernel(**inputs):
    key = str(MM_DT)
    if key not in _CACHE:
        _CACHE[key] = build_kernel(MM_DT)
    nc = _CACHE[key]
    in_maps = _prep_inputs(**inputs)
    r = run_bass_kernel_spmd(nc, in_maps, core_ids=list(range(N_CORES)))
    return _postprocess(r.results)
